# revision 1
# baseline (speedup 1.0000x reference)
"""ChameleonAttention on 8 Trainium2 NeuronCores.

Tensor-parallel over heads: each core owns 4 of the 32 heads.
  - Wq/Wk/Wv sharded column-wise (512 cols/core), Wo row-wise (512 rows/core)
  - per-head LayerNorm + RoPE computed on-chip, gamma/beta replicated
  - causal attention with block-skipping (only lower-triangular key tiles)
  - per-core partial output [S, HID] summed on host (the TP all-reduce)

Precision: projections contract K=4096 in fp16 (full PE rate, ~1e-3 rel
err, halves the operand DMA vs f32 so panel loads hide under compute).
Attention operands + output projection in fp16 (values are O(1) after
LayerNorm / softmax; keeps QT/KT/V resident in SBUF with no DRAM spills).
Softmax uses exp(s*scale - 4) with no running max (LayerNormed q/k bound the
logits), denominator via an all-ones stationary matmul, division deferred to
after the P@V accumulation.

Projection runs in two S-halves so the fp32 accumulators + hidden-state
panels fit in SBUF; PSUM accumulates each K-panel, SBUF fp32 accumulators
carry the full K=4096 contraction. Panels are [2,6,8,8,8] k-tiles: the small
leading panels shrink the cold-start DMA (first matmul group needs only 2
k-tiles of operands, ~0.75MB) while steady-state panels stay at 8.

Causal diagonal blocks use partial-width matmuls: for the k-tile at offset
`toff` inside a 512-query block only queries >= 128*toff can attend, so the
score/PV/denominator matmuls stream N = 512-128*toff columns, and only the
128-wide boundary tile needs the triangular mask.

RoPE is folded with the LayerNorm affine on the host:
  q'[s,j] = xn[s,j]*C1[s,j] + xn[s,p(j)]*D[s,j] + E[s,j]
with C1 = gamma*cos, D = sign*gamma[perm]*sin, E = beta*cos +
sign*beta[perm]*sin, so on-chip RoPE is 4 big DVE ops per 512-wide tile.
"""
import math
from contextlib import ExitStack

import numpy as np

_S = 2048
_HID = 4096
_D = 128
_NC = 8
_CPW = _HID // _NC  # columns per core (512) = 4 heads
_HPC = _CPW // _D  # heads per core (4)
# k-tiles per PSUM accumulation panel, per S-half. Small leading panels in
# half 0 shrink the cold-start DMA; the big last panel stretches each matmul
# group to ~3us so the per-site LN/RoPE chains (~3.3us on DVE) hide under PE
# work between transpose emissions. Half 1 needs no cold-start panels, and
# its 8-tile first panel gives half 0's tail LN sites matmul cover.
_PANELS = ((2, 8, 8, 14), (8, 8, 16))
_ROPE_THETA = 10000.0
_EPS = 1e-5
_EXP_BIAS = -4.0

_cache = {}


def _build(S, niter=1, phases=('p', 'a', 'o'), knobs=None):
    kb = {'proj': 24, 'wpool': 48, 'pps': 6, 'tps': 2, 'upool': 6, 'sps': 4,
          'ops': 2, 'dps': 2, 'lnbig': 4, 'lntmp': 5, 'xps': 6, 'wop': 32}
    kb.update(knobs or {})
    import concourse.tile as tile
    from concourse import bacc, mybir
    from concourse.masks import make_identity

    f32 = mybir.dt.float32
    f16 = mybir.dt.float16
    mul = mybir.AluOpType.mult
    add = mybir.AluOpType.add

    NM = S // 128  # s-tiles (16)
    NQB = S // 512  # query banks (4)
    NMH = NM // 2  # s-tiles per half (8)
    SH = S // 2  # rows per half

    nc = bacc.Bacc("TRN2", target_bir_lowering=False, debug=False)

    hT_d = nc.dram_tensor("hT", [_HID, S], f16, kind="ExternalInput")
    w_d = {
        t: nc.dram_tensor(f"w{t}", [_HID, _CPW], f16, kind="ExternalInput")
        for t in ("q", "k", "v")
    }
    wo_d = nc.dram_tensor("wo", [_CPW, _HID], f16, kind="ExternalInput")
    rope_d = {
        t: nc.dram_tensor(f"rope{t}", [S, 3, _D], f16, kind="ExternalInput")
        for t in ("q", "k")
    }
    masks_d = nc.dram_tensor("masks", [128, 128], f16, kind="ExternalInput")
    out_d = nc.dram_tensor("out", [S, _HID], f16, kind="ExternalOutput")

    for _it in range(niter):
      _p = f'i{_it}_' if niter > 1 else ''
      with tile.TileContext(nc) as tc, ExitStack() as ctx:
          # ---- persistent small constants ----
          persist = ctx.enter_context(tc.tile_pool(name=f"{_p}persist", bufs=1))
          ident16 = persist.tile([128, 128], f16)
          make_identity(nc, ident16[:])
          ones16 = persist.tile([128, 128], f16)
          nc.vector.memset(ones16[:], 1.0)
          ebias = persist.tile([128, 1], f32)
          nc.vector.memset(ebias[:], _EXP_BIAS)
          epst = persist.tile([128, 1], f32)
          nc.vector.memset(epst[:], _EPS)

          # ---- fp16 attention operands, filled by phase P ----
          # quarter-granular tiles (4 s-tiles each): consumers depend only on
          # the quarter they read, so attention can start while the last
          # projection sites are still in flight (deps are tile-granular)
          NQ = NM // 4  # quarters (4)
          att = ctx.enter_context(tc.tile_pool(name=f"{_p}att", bufs=1))
          qtq = [att.tile([128, _HPC, 512], f16, name=f"{_p}qt{i}")
                 for i in range(NQ)]
          ktq = [att.tile([128, _HPC, 512], f16, name=f"{_p}kt{i}")
                 for i in range(NQ)]
          vq = [att.tile([128, 4, 512], f16, name=f"{_p}v{i}")
                for i in range(NQ)]

          # ================= phase P: QKV projection =================
          with ExitStack() as pctx:
              acc_pool = pctx.enter_context(tc.tile_pool(name=f"{_p}acc", bufs=1))
              acc = {}
              for t in ("q", "k", "v"):
                  for m in range(NMH):
                      # f16 inter-panel carry: ~1e-3 rel err, halves SBUF
                      acc[(t, m)] = acc_pool.tile(
                          [128, 512], f16, name=f"{_p}acc_{t}{m}"
                      )
              proj = pctx.enter_context(tc.tile_pool(name=f"{_p}proj", bufs=kb["proj"]))
              wpool = pctx.enter_context(tc.tile_pool(name=f"{_p}wpool", bufs=kb["wpool"]))
              lnbig = pctx.enter_context(tc.tile_pool(name=f"{_p}lnbig", bufs=kb["lnbig"]))
              lntmp = pctx.enter_context(tc.tile_pool(name=f"{_p}lntmp", bufs=kb["lntmp"]))
              pps = pctx.enter_context(tc.tile_pool(name=f"{_p}pps", bufs=kb["pps"], space="PSUM"))
              tps = pctx.enter_context(tc.tile_pool(name=f"{_p}tps", bufs=kb["tps"], space="PSUM"))

              half_d = _D // 2

              def _ln_rope(t, half, m, x16):
                  """LN + RoPE in f16, split across DVE and Pool; transposes
                  are emitted later (deferred past ~2 matmul groups) so the
                  PE FIFO never stalls on this chain."""
                  gm = half * NMH + m
                  c3 = lntmp.tile([128, 3, _D], f16, tag="c3",
                                  name=f"{_p}c3_{half}{t}{m}")
                  nc.sync.dma_start(
                      c3[:], rope_d[t][gm * 128 : (gm + 1) * 128, :, :]
                  )
                  c1, dd, ee = c3[:, 0, :], c3[:, 1, :], c3[:, 2, :]
                  xn4 = lnbig.tile([128, _HPC, _D], f16, tag="xn4",
                                   name=f"{_p}xn4_{half}{t}{m}")
                  for h in range(_HPC):
                      x = x16[:, h * _D : (h + 1) * _D]
                      st = lntmp.tile([128, 6], f32, tag="st",
                                      name=f"{_p}st_{half}{t}{m}{h}")
                      mv = lntmp.tile([128, 2], f32, tag="mv",
                                      name=f"{_p}mv_{half}{t}{m}{h}")
                      nc.vector.bn_stats(out=st[:], in_=x)
                      nc.vector.bn_aggr(out=mv[:], in_=st[:])
                      rstd = lntmp.tile([128, 1], f32, tag="rs",
                                        name=f"{_p}rs_{half}{t}{m}{h}")
                      nc.scalar.activation(
                          out=rstd[:], in_=mv[:, 1:2],
                          func=mybir.ActivationFunctionType.Sqrt,
                          bias=epst[:], scale=1.0,
                      )
                      nc.vector.reciprocal(out=rstd[:], in_=rstd[:])
                      nc.vector.tensor_scalar(
                          out=xn4[:, h, :], in0=x,
                          scalar1=mv[:, 0:1], scalar2=rstd[:],
                          op0=mybir.AluOpType.subtract, op1=mul,
                      )
                  q14 = lnbig.tile([128, _HPC, _D], f16, tag="q14",
                                   name=f"{_p}q14_{half}{t}{m}")
                  t24 = lnbig.tile([128, _HPC, _D], f16, tag="t24",
                                   name=f"{_p}t24_{half}{t}{m}")
                  c1b = c1.unsqueeze(1).broadcast_to((128, _HPC, _D))
                  eeb = ee.unsqueeze(1).broadcast_to((128, _HPC, _D))
                  dd_lo = dd[:, :half_d].unsqueeze(1).broadcast_to(
                      (128, _HPC, half_d))
                  dd_hi = dd[:, half_d:].unsqueeze(1).broadcast_to(
                      (128, _HPC, half_d))
                  # rotate-half folded into sliced multiplies (no copies);
                  # the dd/ee terms run on the otherwise-idle Pool engine
                  nc.gpsimd.tensor_tensor(
                      t24[:, :, :half_d], xn4[:, :, half_d:], dd_lo, op=mul)
                  nc.gpsimd.tensor_tensor(
                      t24[:, :, half_d:], xn4[:, :, :half_d], dd_hi, op=mul)
                  nc.gpsimd.tensor_tensor(t24[:], t24[:], eeb, op=add)
                  nc.vector.tensor_tensor(q14[:], xn4[:], c1b, op=mul)
                  nc.vector.tensor_tensor(q14[:], q14[:], t24[:], op=add)
                  return (t, gm, q14)

              def _emit_tp(t, gm, q14):
                  dst = (qtq if t == "q" else ktq)[gm // 4]
                  g4 = gm % 4
                  tp = tps.tile([128, _HPC, _D], f16, tag="tp",
                                name=f"{_p}tp_{gm}{t}")
                  for h in range(_HPC):
                      nc.tensor.transpose(tp[:, h, :], q14[:, h, :], ident16[:])
                  nc.scalar.copy(dst[:, :, g4 * 128 : (g4 + 1) * 128], tp[:])

              pending = []  # LN'd sites awaiting transpose emission
              for half in range(2):
                  koff = 0
                  for kp, kpt in enumerate(_PANELS[half]):
                      last = kp == len(_PANELS[half]) - 1
                      hts = []
                      for k4 in range(kpt):
                          ht = proj.tile([128, SH], f16, tag="ht",
                                         name=f"{_p}ht_{half}_{kp}_{k4}")
                          kk = koff + k4
                          nc.sync.dma_start(
                              ht[:],
                              hT_d[kk * 128 : (kk + 1) * 128,
                                   half * SH : (half + 1) * SH],
                          )
                          hts.append(ht)
                      wts = {}
                      for t in ("q", "k", "v"):
                          for k4 in range(kpt):
                              wt = wpool.tile([128, 512], f16, tag="w",
                                              name=f"{_p}w{t}_{half}_{kp}_{k4}")
                              kk = koff + k4
                              nc.sync.dma_start(
                                  wt[:], w_d[t][kk * 128 : (kk + 1) * 128, :]
                              )
                              wts[(t, k4)] = wt

                      def _group(t, m):
                          ps = pps.tile([128, 512], f32, tag="ps",
                                        name=f"{_p}ps_{half}_{kp}_{t}_{m}")
                          for k4 in range(kpt):
                              nc.tensor.matmul(
                                  ps[:],
                                  hts[k4][:, m * 128 : (m + 1) * 128],
                                  wts[(t, k4)][:],
                                  start=(k4 == 0),
                                  stop=(k4 == kpt - 1),
                              )
                          return ps

                      if not last:
                          for t in ("q", "k", "v"):
                              for m in range(NMH):
                                  ps = _group(t, m)
                                  if kp == 0:
                                      nc.vector.tensor_copy(acc[(t, m)][:], ps[:])
                                  else:
                                      nc.vector.tensor_tensor(
                                          acc[(t, m)][:], acc[(t, m)][:], ps[:],
                                          op=add,
                                      )
                                  if len(pending) > 2:
                                      _emit_tp(*pending.pop(0))
                      else:
                          # final panel: interleave q/k/v per m-row so the
                          # 2 LN sites per row hide under 3 matmul groups
                          for m in range(NMH):
                              gm = half * NMH + m
                              for t in ("q", "k", "v"):
                                  ps = _group(t, m)
                                  if t == "v":
                                      # fuse final add + fp16 cast
                                      nc.vector.tensor_tensor(
                                          vq[gm // 4][:, gm % 4, :],
                                          acc[(t, m)][:], ps[:],
                                          op=add,
                                      )
                                  else:
                                      # fused add + f16 cast feeds LN at 2x
                                      # DVE rate; acc is dead after this
                                      x16 = lnbig.tile(
                                          [128, 512], f16, tag="x16",
                                          name=f"{_p}x16_{half}{t}{m}")
                                      nc.vector.tensor_tensor(
                                          x16[:], acc[(t, m)][:], ps[:], op=add)
                                      pending.append(_ln_rope(t, half, m, x16))
                                  if len(pending) > 2:
                                      _emit_tp(*pending.pop(0))
                      koff += kpt
              while pending:
                  _emit_tp(*pending.pop(0))

          # ============ phases A+O share at_t ============
          if 'a' not in phases and 'o' not in phases:
              continue
          with ExitStack() as aoctx:
              aop = aoctx.enter_context(tc.tile_pool(name=f"{_p}aop", bufs=1))
              atq = [aop.tile([128, _HPC, 512], f16, name=f"{_p}at{i}")
                     for i in range(NQ)]

              # mask first (needed at phase-A start), then prefetch all of Wo
              # during attention while the DMA engines are otherwise idle
              mpool = aoctx.enter_context(tc.tile_pool(name=f"{_p}mpool", bufs=1))
              mask_t = mpool.tile([128, 128], f16)
              if 'a' in phases:
                  nc.sync.dma_start(mask_t[:], masks_d.ap())
              wop = aoctx.enter_context(tc.tile_pool(name=f"{_p}wop", bufs=kb["wop"]))
              NB = _HID // 512  # output column blocks (8)
              wo_tiles = {}
              if 'o' in phases:
                  for n in range(NB):
                      for k4 in range(_HPC):
                          wo_t = wop.tile([128, 512], f16, tag="wo",
                                          name=f"{_p}wo_{n}_{k4}")
                          nc.sync.dma_start(
                              wo_t[:],
                              wo_d[k4 * 128 : (k4 + 1) * 128,
                                   n * 512 : (n + 1) * 512],
                          )
                          wo_tiles[(n, k4)] = wo_t

              # ---------- phase A: causal attention ----------
              # qb-outer: at4 columns complete in m order so the output
              # projection's operands are ready long before phase O starts
              with ExitStack() as actx:
                  if 'a' in phases:
                      upool = actx.enter_context(tc.tile_pool(name=f"{_p}upool", bufs=kb["upool"]))
                      rpool = actx.enter_context(tc.tile_pool(name=f"{_p}rpool", bufs=2))
                      sps = actx.enter_context(tc.tile_pool(name=f"{_p}sps", bufs=kb["sps"], space="PSUM"))
                      ops = actx.enter_context(tc.tile_pool(name=f"{_p}ops", bufs=kb["ops"], space="PSUM"))
                      dps = actx.enter_context(tc.tile_pool(name=f"{_p}dps", bufs=kb["dps"], space="PSUM"))

                      scale = 1.0 / math.sqrt(_D)
                      for qb in range(NQB):
                          for h in range(_HPC):
                              o_ps = ops.tile([128, 512], f32, tag="o", name=f"{_p}o_{h}_{qb}")
                              d_ps = dps.tile([128, 512], f32, tag="d", name=f"{_p}d_{h}_{qb}")
                              nkt = 4 * qb + 4
                              for kt in range(nkt):
                                  # causal: queries < kt*128 can't see this
                                  # k-tile; stream only columns [off:512)
                                  toff = kt - 4 * qb
                                  off = 128 * toff if toff > 0 else 0
                                  s_ps = sps.tile([128, 512], f32, tag="s",
                                                  name=f"{_p}s_{h}_{qb}_{kt}")
                                  k4i = kt % 4
                                  nc.tensor.matmul(
                                      s_ps[:, off:],
                                      ktq[kt // 4][:, h, k4i * 128 : (k4i + 1) * 128],
                                      qtq[qb][:, h, off:],
                                      start=True, stop=True,
                                  )
                                  u = upool.tile([128, 512], f16, tag="u",
                                                 name=f"{_p}u_{h}_{qb}_{kt}")
                                  nc.scalar.activation(
                                      out=u[:, off:], in_=s_ps[:, off:],
                                      func=mybir.ActivationFunctionType.Exp,
                                      bias=ebias[:], scale=scale,
                                  )
                                  if toff >= 0:
                                      nc.vector.tensor_tensor(
                                          u[:, off : off + 128],
                                          u[:, off : off + 128],
                                          mask_t[:], op=mul,
                                      )
                                  nc.tensor.matmul(
                                      o_ps[:, off:],
                                      vq[kt // 4][:, kt % 4, h * _D : (h + 1) * _D],
                                      u[:, off:],
                                      start=(kt == 0), stop=(kt == nkt - 1),
                                  )
                                  nc.tensor.matmul(
                                      d_ps[:, off:], ones16[:], u[:, off:],
                                      start=(kt == 0), stop=(kt == nkt - 1),
                                  )
                              rec = rpool.tile([128, 512], f32, tag="r", name=f"{_p}r_{h}_{qb}")
                              nc.vector.reciprocal(out=rec[:], in_=d_ps[:])
                              nc.vector.tensor_tensor(
                                  atq[qb][:, h, :], o_ps[:], rec[:],
                                  op=mul,
                              )

              # ---------- phase O: output projection ----------
              with ExitStack() as octx:
                  if 'o' in phases:
                      outp = octx.enter_context(tc.tile_pool(name=f"{_p}outp", bufs=2))
                      xps = octx.enter_context(tc.tile_pool(name=f"{_p}xps", bufs=kb["xps"], space="PSUM"))

                      for n in range(NB):
                          for m in range(NM):
                              mi = m % 4
                              ps = xps.tile([128, 512], f32, tag="x", name=f"{_p}x_{n}_{m}")
                              for k4 in range(_HPC):
                                  nc.tensor.matmul(
                                      ps[:],
                                      atq[m // 4][:, k4, mi * 128 : (mi + 1) * 128],
                                      wo_tiles[(n, k4)][:],
                                      start=(k4 == 0), stop=(k4 == _HPC - 1),
                                  )
                              ot = outp.tile([128, 512], f16, tag="ot",
                                             name=f"{_p}ot_{n}_{m}", bufs=6)
                              nc.vector.tensor_copy(ot[:], ps[:])
                              nc.sync.dma_start(
                                  out_d[m * 128 : (m + 1) * 128,
                                        n * 512 : (n + 1) * 512],
                                  ot[:],
                              )


    nc.compile()
    return nc


def _host_prep(hidden_states, position_ids, Wq, Wk, Wv, Wo, qn_w, qn_b, kn_w, kn_b):
    S = hidden_states.shape[1]
    hT = np.ascontiguousarray(
        np.asarray(hidden_states, np.float32)[0].T.astype(np.float16)
    )
    pos = np.asarray(position_ids, np.float32)[0]  # [S]
    inv = 1.0 / (_ROPE_THETA ** (np.arange(0, _D, 2, dtype=np.float32) / _D))
    fr = pos[:, None] * inv[None, :]  # [S, D/2]
    emb = np.concatenate([fr, fr], axis=1)  # [S, D]
    cos = np.cos(emb).astype(np.float32)
    sin = np.sin(emb).astype(np.float32)

    half = _D // 2
    perm = np.concatenate([np.arange(half, _D), np.arange(0, half)])
    sign = np.concatenate([-np.ones(half, np.float32), np.ones(half, np.float32)])

    def coeffs(g, b):
        g = np.asarray(g, np.float32).reshape(_D)
        b = np.asarray(b, np.float32).reshape(_D)
        c1 = g[None, :] * cos  # [S, D]
        dd = (sign * g[perm])[None, :] * sin
        ee = b[None, :] * cos + (sign * b[perm])[None, :] * sin
        return c1.astype(np.float16), dd.astype(np.float16), ee.astype(np.float16)

    ropeq = np.ascontiguousarray(np.stack(coeffs(qn_w, qn_b), axis=1))
    ropek = np.ascontiguousarray(np.stack(coeffs(kn_w, kn_b), axis=1))

    kk = np.arange(128)[:, None]
    qq = np.arange(128)[None, :]
    masks = (kk <= qq).astype(np.float16)  # [128, 128] lower-tri in (k, q)

    common = {
        "hT": hT,
        "ropeq": ropeq, "ropek": ropek,
        "masks": masks,
    }
    Wq = np.asarray(Wq, np.float32).astype(np.float16)
    Wk = np.asarray(Wk, np.float32).astype(np.float16)
    Wv = np.asarray(Wv, np.float32).astype(np.float16)
    Wo16 = np.asarray(Wo, np.float32).astype(np.float16)
    in_maps = []
    for c in range(_NC):
        sl = slice(c * _CPW, (c + 1) * _CPW)
        m = dict(common)
        m["wq"] = np.ascontiguousarray(Wq[:, sl])
        m["wk"] = np.ascontiguousarray(Wk[:, sl])
        m["wv"] = np.ascontiguousarray(Wv[:, sl])
        m["wo"] = np.ascontiguousarray(Wo16[sl, :])
        in_maps.append(m)
    return in_maps


def kernel(**inputs) -> np.ndarray:
    from concourse.bass_utils import run_bass_kernel_spmd

    hidden_states = np.asarray(inputs["hidden_states"])
    S = hidden_states.shape[1]
    if S not in _cache:
        _cache[S] = _build(S)
    nc = _cache[S]

    in_maps = _host_prep(
        hidden_states,
        inputs["position_ids"],
        inputs["Wq"], inputs["Wk"], inputs["Wv"], inputs["Wo"],
        inputs["qn_w"], inputs["qn_b"], inputs["kn_w"], inputs["kn_b"],
    )
    res = run_bass_kernel_spmd(nc, in_maps, list(range(_NC)))
    out = np.zeros((S, _HID), np.float32)
    for c in range(_NC):
        out += res.results[c]["out"].astype(np.float32)
    return out.reshape(1, S, _HID)



# revision 15
# speedup vs baseline: 5.0215x; 5.0215x over previous
"""ChameleonAttention on 8 Trainium2 NeuronCores.

Tensor-parallel over heads: each core owns 4 of the 32 heads.
  - Wq/Wk/Wv sharded column-wise (512 cols/core), Wo row-wise (512 rows/core)
  - per-head LayerNorm + RoPE computed on-chip, gamma/beta replicated
  - causal attention with block-skipping (only lower-triangular key tiles)
  - per-core partial output [S, HID] summed on host (the TP all-reduce)

Precision: projections contract K=4096 in fp16 (full PE rate, ~1e-3 rel
err, halves the operand DMA vs f32 so panel loads hide under compute).
Attention operands + output projection in fp16 (values are O(1) after
LayerNorm / softmax; keeps QT/KT/V resident in SBUF with no DRAM spills).
Softmax uses exp(s*scale - 4) with no running max (LayerNormed q/k bound the
logits), denominator via an all-ones stationary matmul, division deferred to
after the P@V accumulation.

Projection runs in two S-halves so the fp32 accumulators + hidden-state
panels fit in SBUF; PSUM accumulates each K-panel, SBUF fp32 accumulators
carry the full K=4096 contraction. Panels are [2,6,8,8,8] k-tiles: the small
leading panels shrink the cold-start DMA (first matmul group needs only 2
k-tiles of operands, ~0.75MB) while steady-state panels stay at 8.

Causal diagonal blocks use partial-width matmuls: for the k-tile at offset
`toff` inside a 512-query block only queries >= 128*toff can attend, so the
score/PV/denominator matmuls stream N = 512-128*toff columns, and only the
128-wide boundary tile needs the triangular mask.

RoPE is folded with the LayerNorm affine on the host:
  q'[s,j] = xn[s,j]*C1[s,j] + xn[s,p(j)]*D[s,j] + E[s,j]
with C1 = gamma*cos, D = sign*gamma[perm]*sin, E = beta*cos +
sign*beta[perm]*sin, so on-chip RoPE is 4 big DVE ops per 512-wide tile.
"""
import math
from contextlib import ExitStack

import numpy as np

_S = 2048
_HID = 4096
_D = 128
_NC = 8
_CPW = _HID // _NC  # columns per core (512) = 4 heads
_HPC = _CPW // _D  # heads per core (4)
# k-tiles per PSUM accumulation panel, per S-half. Small leading panels in
# half 0 shrink the cold-start DMA; the big last panel stretches each matmul
# group to ~3us so the per-site LN/RoPE chains (~3.3us on DVE) hide under PE
# work between transpose emissions. Half 1 needs no cold-start panels, and
# its 8-tile first panel gives half 0's tail LN sites matmul cover.
_PANELS = ((2, 8, 8, 14), (8, 8, 16))
_ROPE_THETA = 10000.0
_EPS = 1e-5
_EXP_BIAS = -4.0

_cache = {}


def _build(S, niter=1, phases=('p', 'a', 'o'), knobs=None):
    kb = {'proj': 24, 'wpool': 16, 'pps': 6, 'tps': 2, 'upool': 6, 'sps': 3,
          'ops': 2, 'dps': 1, 'lnbig': 4, 'lntmp': 5, 'xps': 2, 'wop': 32}
    kb.update(knobs or {})
    import concourse.tile as tile
    from concourse import bacc, mybir
    from concourse.masks import make_identity

    f32 = mybir.dt.float32
    f16 = mybir.dt.float16
    mul = mybir.AluOpType.mult
    add = mybir.AluOpType.add

    NM = S // 128  # s-tiles (16)
    NQB = S // 512  # query banks (4)
    NMH = NM // 2  # s-tiles per half (8)
    SH = S // 2  # rows per half

    nc = bacc.Bacc("TRN2", target_bir_lowering=False, debug=False)

    hT_d = nc.dram_tensor("hT", [_HID, S], f16, kind="ExternalInput")
    # q|k|v weights concatenated column-wise: one DMA per k-tile (3KB DRAM
    # lines) instead of three
    wqkv_d = nc.dram_tensor("wqkv", [_HID, 3 * _CPW], f16, kind="ExternalInput")
    wo_d = nc.dram_tensor("wo", [_CPW, _HID], f16, kind="ExternalInput")
    rope_d = {
        t: nc.dram_tensor(f"rope{t}", [S, 3, _D], f16, kind="ExternalInput")
        for t in ("q", "k")
    }
    masks_d = nc.dram_tensor("masks", [128, 128], f16, kind="ExternalInput")
    out_d = nc.dram_tensor("out", [S, _HID], f16, kind="ExternalOutput")

    for _it in range(niter):
      _p = f'i{_it}_' if niter > 1 else ''
      with tile.TileContext(nc) as tc, ExitStack() as ctx:
          # ---- persistent small constants ----
          persist = ctx.enter_context(tc.tile_pool(name=f"{_p}persist", bufs=1))
          ident16 = persist.tile([128, 128], f16)
          make_identity(nc, ident16[:])
          ones16 = persist.tile([128, 128], f16)
          nc.vector.memset(ones16[:], 1.0)
          ebias = persist.tile([128, 1], f32)
          nc.vector.memset(ebias[:], _EXP_BIAS)
          epst = persist.tile([128, 1], f32)
          nc.vector.memset(epst[:], _EPS)

          # ---- fp16 attention operands, filled by phase P ----
          # quarter-granular tiles (4 s-tiles each): consumers depend only on
          # the quarter they read, so attention can start while the last
          # projection sites are still in flight (deps are tile-granular)
          NQ = NM // 4  # quarters (4)
          att = ctx.enter_context(tc.tile_pool(name=f"{_p}att", bufs=1))
          qtq = [att.tile([128, _HPC, 512], f16, name=f"{_p}qt{i}")
                 for i in range(NQ)]
          ktq = [att.tile([128, _HPC, 512], f16, name=f"{_p}kt{i}")
                 for i in range(NQ)]
          vq = [att.tile([128, 4, 512], f16, name=f"{_p}v{i}")
                for i in range(NQ)]

          # ================= phase P: QKV projection =================
          with ExitStack() as pctx:
              acc_pool = pctx.enter_context(tc.tile_pool(name=f"{_p}acc", bufs=1))
              acc = {}
              for t in ("q", "k", "v"):
                  for m in range(NMH):
                      # f16 inter-panel carry: ~1e-3 rel err, halves SBUF
                      acc[(t, m)] = acc_pool.tile(
                          [128, 512], f16, name=f"{_p}acc_{t}{m}"
                      )
              proj = pctx.enter_context(tc.tile_pool(name=f"{_p}proj", bufs=kb["proj"]))
              wpool = pctx.enter_context(tc.tile_pool(name=f"{_p}wpool", bufs=kb["wpool"]))
              lnbig = pctx.enter_context(tc.tile_pool(name=f"{_p}lnbig", bufs=kb["lnbig"]))
              lntmp = pctx.enter_context(tc.tile_pool(name=f"{_p}lntmp", bufs=kb["lntmp"]))
              pps = pctx.enter_context(tc.tile_pool(name=f"{_p}pps", bufs=kb["pps"], space="PSUM"))
              tps = pctx.enter_context(tc.tile_pool(name=f"{_p}tps", bufs=kb["tps"], space="PSUM"))

              half_d = _D // 2

              def _ln_rope(t, half, m, x16):
                  """LN + RoPE in f16, split across DVE and Pool; transposes
                  are emitted later (deferred past ~2 matmul groups) so the
                  PE FIFO never stalls on this chain."""
                  gm = half * NMH + m
                  c3 = lntmp.tile([128, 3, _D], f16, tag="c3",
                                  name=f"{_p}c3_{half}{t}{m}")
                  nc.sync.dma_start(
                      c3[:], rope_d[t][gm * 128 : (gm + 1) * 128, :, :]
                  )
                  c1, dd, ee = c3[:, 0, :], c3[:, 1, :], c3[:, 2, :]
                  xn4 = lnbig.tile([128, _HPC, _D], f16, tag="xn4",
                                   name=f"{_p}xn4_{half}{t}{m}")
                  for h in range(_HPC):
                      x = x16[:, h * _D : (h + 1) * _D]
                      st = lntmp.tile([128, 6], f32, tag="st",
                                      name=f"{_p}st_{half}{t}{m}{h}")
                      mv = lntmp.tile([128, 2], f32, tag="mv",
                                      name=f"{_p}mv_{half}{t}{m}{h}")
                      nc.vector.bn_stats(out=st[:], in_=x)
                      nc.vector.bn_aggr(out=mv[:], in_=st[:])
                      rstd = lntmp.tile([128, 1], f32, tag="rs",
                                        name=f"{_p}rs_{half}{t}{m}{h}")
                      nc.scalar.activation(
                          out=rstd[:], in_=mv[:, 1:2],
                          func=mybir.ActivationFunctionType.Sqrt,
                          bias=epst[:], scale=1.0,
                      )
                      nc.vector.reciprocal(out=rstd[:], in_=rstd[:])
                      nc.vector.tensor_scalar(
                          out=xn4[:, h, :], in0=x,
                          scalar1=mv[:, 0:1], scalar2=rstd[:],
                          op0=mybir.AluOpType.subtract, op1=mul,
                      )
                  q14 = lnbig.tile([128, _HPC, _D], f16, tag="q14",
                                   name=f"{_p}q14_{half}{t}{m}")
                  t24 = lnbig.tile([128, _HPC, _D], f16, tag="t24",
                                   name=f"{_p}t24_{half}{t}{m}")
                  c1b = c1.unsqueeze(1).broadcast_to((128, _HPC, _D))
                  eeb = ee.unsqueeze(1).broadcast_to((128, _HPC, _D))
                  dd_lo = dd[:, :half_d].unsqueeze(1).broadcast_to(
                      (128, _HPC, half_d))
                  dd_hi = dd[:, half_d:].unsqueeze(1).broadcast_to(
                      (128, _HPC, half_d))
                  # rotate-half folded into sliced multiplies (no copies);
                  # the dd/ee terms run on the otherwise-idle Pool engine
                  nc.gpsimd.tensor_tensor(
                      t24[:, :, :half_d], xn4[:, :, half_d:], dd_lo, op=mul)
                  nc.gpsimd.tensor_tensor(
                      t24[:, :, half_d:], xn4[:, :, :half_d], dd_hi, op=mul)
                  nc.gpsimd.tensor_tensor(t24[:], t24[:], eeb, op=add)
                  nc.vector.tensor_tensor(q14[:], xn4[:], c1b, op=mul)
                  nc.vector.tensor_tensor(q14[:], q14[:], t24[:], op=add)
                  return (t, gm, q14)

              def _emit_tp(t, gm, q14):
                  dst = (qtq if t == "q" else ktq)[gm // 4]
                  g4 = gm % 4
                  tp = tps.tile([128, _HPC, _D], f16, tag="tp",
                                name=f"{_p}tp_{gm}{t}")
                  for h in range(_HPC):
                      nc.tensor.transpose(tp[:, h, :], q14[:, h, :], ident16[:])
                  nc.scalar.copy(dst[:, :, g4 * 128 : (g4 + 1) * 128], tp[:])

              pending = []  # LN'd sites awaiting transpose emission
              for half in range(2):
                  koff = 0
                  for kp, kpt in enumerate(_PANELS[half]):
                      last = kp == len(_PANELS[half]) - 1
                      cold = half == 0 and kp == 0
                      hts = []
                      wts = {}
                      if cold:
                          # critical-path-ordered issue: the first matmul
                          # group (t=q, m=0) needs only ht[:, :512] and the
                          # q-plane of each w tile; land those first
                          wtiles = []
                          for k4 in range(kpt):
                              ht = proj.tile([128, SH], f16, tag="ht",
                                             name=f"{_p}ht_{half}_{kp}_{k4}")
                              kk = koff + k4
                              src = hT_d[kk * 128 : (kk + 1) * 128,
                                         half * SH : (half + 1) * SH]
                              hh = SH // 2
                              nc.sync.dma_start(ht[:, :hh], src[:, :hh])
                              hts.append(ht)
                              wt = wpool.tile([128, 3, 512], f16, tag="w",
                                              name=f"{_p}w_{half}_{kp}_{k4}")
                              wsrc = wqkv_d[kk * 128 : (kk + 1) * 128, :]
                              nc.sync.dma_start(wt[:, 0, :], wsrc[:, :512])
                              wtiles.append((wt, wsrc))
                              wts[("q", k4)] = wt[:, 0, :]
                          for k4 in range(kpt):
                              kk = koff + k4
                              hh = SH // 2
                              nc.sync.dma_start(
                                  hts[k4][:, hh:],
                                  hT_d[kk * 128 : (kk + 1) * 128,
                                       half * SH + hh : (half + 1) * SH],
                              )
                              wt, wsrc = wtiles[k4]
                              for ti, t in enumerate(("k", "v"), start=1):
                                  nc.sync.dma_start(
                                      wt[:, ti, :],
                                      wsrc[:, ti * 512 : (ti + 1) * 512],
                                  )
                                  wts[(t, k4)] = wt[:, ti, :]
                      else:
                          for k4 in range(kpt):
                              ht = proj.tile([128, SH], f16, tag="ht",
                                             name=f"{_p}ht_{half}_{kp}_{k4}")
                              kk = koff + k4
                              nc.sync.dma_start(
                                  ht[:],
                                  hT_d[kk * 128 : (kk + 1) * 128,
                                       half * SH : (half + 1) * SH],
                              )
                              hts.append(ht)
                              wt = wpool.tile([128, 3, 512], f16, tag="w",
                                              name=f"{_p}w_{half}_{kp}_{k4}")
                              nc.sync.dma_start(
                                  wt[:], wqkv_d[kk * 128 : (kk + 1) * 128, :]
                              )
                              for ti, t in enumerate(("q", "k", "v")):
                                  wts[(t, k4)] = wt[:, ti, :]

                      def _group(t, m):
                          ps = pps.tile([128, 512], f32, tag="ps",
                                        name=f"{_p}ps_{half}_{kp}_{t}_{m}")
                          for k4 in range(kpt):
                              nc.tensor.matmul(
                                  ps[:],
                                  hts[k4][:, m * 128 : (m + 1) * 128],
                                  wts[(t, k4)],
                                  start=(k4 == 0),
                                  stop=(k4 == kpt - 1),
                              )
                          return ps

                      if not last:
                          for t in ("q", "k", "v"):
                              for m in range(NMH):
                                  ps = _group(t, m)
                                  if kp == 0:
                                      nc.vector.tensor_copy(acc[(t, m)][:], ps[:])
                                  else:
                                      nc.vector.tensor_tensor(
                                          acc[(t, m)][:], acc[(t, m)][:], ps[:],
                                          op=add,
                                      )
                                  if len(pending) > 2:
                                      _emit_tp(*pending.pop(0))
                      else:
                          # final panel: interleave q/k/v per m-row so the
                          # 2 LN sites per row hide under 3 matmul groups
                          for m in range(NMH):
                              gm = half * NMH + m
                              for t in ("q", "k", "v"):
                                  ps = _group(t, m)
                                  if t == "v":
                                      # fuse final add + fp16 cast
                                      nc.vector.tensor_tensor(
                                          vq[gm // 4][:, gm % 4, :],
                                          acc[(t, m)][:], ps[:],
                                          op=add,
                                      )
                                  else:
                                      # fused add + f16 cast feeds LN at 2x
                                      # DVE rate; acc is dead after this
                                      x16 = lnbig.tile(
                                          [128, 512], f16, tag="x16",
                                          name=f"{_p}x16_{half}{t}{m}")
                                      nc.vector.tensor_tensor(
                                          x16[:], acc[(t, m)][:], ps[:], op=add)
                                      pending.append(_ln_rope(t, half, m, x16))
                                  if len(pending) > 2:
                                      _emit_tp(*pending.pop(0))
                      koff += kpt
              # dummy exp after the last LN Sqrt: the Sqrt->Exp act-table
              # swap (1.28us) runs under the projection tail matmuls instead
              # of stalling attention's first softmax
              warm = lntmp.tile([128, 1], f32, tag="warm", name=f"{_p}warm")
              nc.scalar.activation(
                  out=warm[:], in_=epst[:],
                  func=mybir.ActivationFunctionType.Exp,
                  bias=ebias[:], scale=1.0,
              )
              while pending:
                  _emit_tp(*pending.pop(0))

          # ============ phases A+O share at_t ============
          if 'a' not in phases and 'o' not in phases:
              continue
          with ExitStack() as aoctx:
              aop = aoctx.enter_context(tc.tile_pool(name=f"{_p}aop", bufs=1))
              atq = [aop.tile([128, _HPC, 512], f16, name=f"{_p}at{i}")
                     for i in range(NQ)]

              # mask first (needed at phase-A start), then prefetch all of Wo
              # during attention while the DMA engines are otherwise idle
              mpool = aoctx.enter_context(tc.tile_pool(name=f"{_p}mpool", bufs=1))
              mask_t = mpool.tile([128, 128], f16)
              if 'a' in phases:
                  nc.sync.dma_start(mask_t[:], masks_d.ap())
              wop = aoctx.enter_context(tc.tile_pool(name=f"{_p}wop", bufs=kb["wop"]))
              NB = _HID // 512  # output column blocks (8)
              wo_tiles = {}
              if 'o' in phases:
                  for n in range(NB):
                      for k4 in range(_HPC):
                          wo_t = wop.tile([128, 512], f16, tag="wo",
                                          name=f"{_p}wo_{n}_{k4}")
                          nc.sync.dma_start(
                              wo_t[:],
                              wo_d[k4 * 128 : (k4 + 1) * 128,
                                   n * 512 : (n + 1) * 512],
                          )
                          wo_tiles[(n, k4)] = wo_t

              # A and O PSUM pools coexist (3+2+1+2 = 8 banks): phase O's
              # first matmul waits only on its own banks, not on the full
              # attention PSUM drain
              if 'a' in phases:
                  sps = aoctx.enter_context(tc.tile_pool(name=f"{_p}sps", bufs=kb["sps"], space="PSUM"))
                  ops = aoctx.enter_context(tc.tile_pool(name=f"{_p}ops", bufs=kb["ops"], space="PSUM"))
                  dps = aoctx.enter_context(tc.tile_pool(name=f"{_p}dps", bufs=kb["dps"], space="PSUM"))
              if 'o' in phases:
                  xps = aoctx.enter_context(tc.tile_pool(name=f"{_p}xps", bufs=kb["xps"], space="PSUM"))

              # ---------- phase A: causal attention ----------
              # qb-outer: at4 columns complete in m order so the output
              # projection's operands are ready long before phase O starts
              # (pools live in aoctx: an ExitStack close between A and O
              # would emit a full engine barrier and stall PE ~2us)
              if True:
                  if 'a' in phases:
                      upool = aoctx.enter_context(tc.tile_pool(name=f"{_p}upool", bufs=kb["upool"]))
                      rpool = aoctx.enter_context(tc.tile_pool(name=f"{_p}rpool", bufs=2))

                      scale = 1.0 / math.sqrt(_D)
                      for qb in range(NQB):
                          for h in range(_HPC):
                              o_ps = ops.tile([128, 512], f32, tag="o", name=f"{_p}o_{h}_{qb}")
                              d_ps = dps.tile([128, 512], f32, tag="d", name=f"{_p}d_{h}_{qb}")
                              nkt = 4 * qb + 4
                              for kt in range(nkt):
                                  # causal: queries < kt*128 can't see this
                                  # k-tile; stream only columns [off:512)
                                  toff = kt - 4 * qb
                                  off = 128 * toff if toff > 0 else 0
                                  s_ps = sps.tile([128, 512], f32, tag="s",
                                                  name=f"{_p}s_{h}_{qb}_{kt}")
                                  k4i = kt % 4
                                  nc.tensor.matmul(
                                      s_ps[:, off:],
                                      ktq[kt // 4][:, h, k4i * 128 : (k4i + 1) * 128],
                                      qtq[qb][:, h, off:],
                                      start=True, stop=True,
                                  )
                                  u = upool.tile([128, 512], f16, tag="u",
                                                 name=f"{_p}u_{h}_{qb}_{kt}")
                                  nc.scalar.activation(
                                      out=u[:, off:], in_=s_ps[:, off:],
                                      func=mybir.ActivationFunctionType.Exp,
                                      bias=ebias[:], scale=scale,
                                  )
                                  if toff >= 0:
                                      nc.vector.tensor_tensor(
                                          u[:, off : off + 128],
                                          u[:, off : off + 128],
                                          mask_t[:], op=mul,
                                      )
                                  nc.tensor.matmul(
                                      o_ps[:, off:],
                                      vq[kt // 4][:, kt % 4, h * _D : (h + 1) * _D],
                                      u[:, off:],
                                      start=(kt == 0), stop=(kt == nkt - 1),
                                  )
                                  nc.tensor.matmul(
                                      d_ps[:, off:], ones16[:], u[:, off:],
                                      start=(kt == 0), stop=(kt == nkt - 1),
                                  )
                              rec = rpool.tile([128, 512], f32, tag="r", name=f"{_p}r_{h}_{qb}")
                              nc.vector.reciprocal(out=rec[:], in_=d_ps[:])
                              nc.vector.tensor_tensor(
                                  atq[qb][:, h, :], o_ps[:], rec[:],
                                  op=mul,
                              )

              # ---------- phase O: output projection ----------
              # m-outer: full [128, HID] output rows assemble in SBUF and
              # leave in 4KB-line DMA chunks (32 DMAs, not 128)
              if True:
                  if 'o' in phases:
                      outp = aoctx.enter_context(tc.tile_pool(name=f"{_p}outp", bufs=2))

                      for m in range(NM):
                          mi = m % 4
                          ot = outp.tile([128, _HID], f16, tag="ot",
                                         name=f"{_p}ot_{m}")
                          for n in range(NB):
                              ps = xps.tile([128, 512], f32, tag="x", name=f"{_p}x_{n}_{m}")
                              for k4 in range(_HPC):
                                  nc.tensor.matmul(
                                      ps[:],
                                      atq[m // 4][:, k4, mi * 128 : (mi + 1) * 128],
                                      wo_tiles[(n, k4)][:],
                                      start=(k4 == 0), stop=(k4 == _HPC - 1),
                                  )
                              # copy on Act: DVE is backlogged with the
                              # attention rec/mult chain at the A->O seam,
                              # which starved the 2-bank xps ping-pong
                              nc.scalar.copy(
                                  ot[:, n * 512 : (n + 1) * 512], ps[:]
                              )
                              if n == NB // 2 - 1:
                                  nc.sync.dma_start(
                                      out_d[m * 128 : (m + 1) * 128, : _HID // 2],
                                      ot[:, : _HID // 2],
                                  )
                          nc.sync.dma_start(
                              out_d[m * 128 : (m + 1) * 128, _HID // 2 :],
                              ot[:, _HID // 2 :],
                          )


    nc.compile()
    return nc


def _host_prep(hidden_states, position_ids, Wq, Wk, Wv, Wo, qn_w, qn_b, kn_w, kn_b):
    S = hidden_states.shape[1]
    hT = np.ascontiguousarray(
        np.asarray(hidden_states, np.float32)[0].T.astype(np.float16)
    )
    pos = np.asarray(position_ids, np.float32)[0]  # [S]
    inv = 1.0 / (_ROPE_THETA ** (np.arange(0, _D, 2, dtype=np.float32) / _D))
    fr = pos[:, None] * inv[None, :]  # [S, D/2]
    emb = np.concatenate([fr, fr], axis=1)  # [S, D]
    cos = np.cos(emb).astype(np.float32)
    sin = np.sin(emb).astype(np.float32)

    half = _D // 2
    perm = np.concatenate([np.arange(half, _D), np.arange(0, half)])
    sign = np.concatenate([-np.ones(half, np.float32), np.ones(half, np.float32)])

    def coeffs(g, b):
        g = np.asarray(g, np.float32).reshape(_D)
        b = np.asarray(b, np.float32).reshape(_D)
        c1 = g[None, :] * cos  # [S, D]
        dd = (sign * g[perm])[None, :] * sin
        ee = b[None, :] * cos + (sign * b[perm])[None, :] * sin
        return c1.astype(np.float16), dd.astype(np.float16), ee.astype(np.float16)

    ropeq = np.ascontiguousarray(np.stack(coeffs(qn_w, qn_b), axis=1))
    ropek = np.ascontiguousarray(np.stack(coeffs(kn_w, kn_b), axis=1))

    kk = np.arange(128)[:, None]
    qq = np.arange(128)[None, :]
    masks = (kk <= qq).astype(np.float16)  # [128, 128] lower-tri in (k, q)

    common = {
        "hT": hT,
        "ropeq": ropeq, "ropek": ropek,
        "masks": masks,
    }
    Wq = np.asarray(Wq, np.float32).astype(np.float16)
    Wk = np.asarray(Wk, np.float32).astype(np.float16)
    Wv = np.asarray(Wv, np.float32).astype(np.float16)
    Wo16 = np.asarray(Wo, np.float32).astype(np.float16)
    in_maps = []
    for c in range(_NC):
        sl = slice(c * _CPW, (c + 1) * _CPW)
        m = dict(common)
        m["wqkv"] = np.ascontiguousarray(
            np.concatenate([Wq[:, sl], Wk[:, sl], Wv[:, sl]], axis=1)
        )
        m["wo"] = np.ascontiguousarray(Wo16[sl, :])
        in_maps.append(m)
    return in_maps


def kernel(**inputs) -> np.ndarray:
    from concourse.bass_utils import run_bass_kernel_spmd

    hidden_states = np.asarray(inputs["hidden_states"])
    S = hidden_states.shape[1]
    if S not in _cache:
        _cache[S] = _build(S)
    nc = _cache[S]

    in_maps = _host_prep(
        hidden_states,
        inputs["position_ids"],
        inputs["Wq"], inputs["Wk"], inputs["Wv"], inputs["Wo"],
        inputs["qn_w"], inputs["qn_b"], inputs["kn_w"], inputs["kn_b"],
    )
    res = run_bass_kernel_spmd(nc, in_maps, list(range(_NC)))
    out = np.zeros((S, _HID), np.float32)
    for c in range(_NC):
        out += res.results[c]["out"].astype(np.float32)
    return out.reshape(1, S, _HID)

